# revision 55
# baseline (speedup 1.0000x reference)
"""Gated GQA attention block (B=2,S=2048,E=2048,H=16,HKV=2,D=256,RD=64) on 8 TRN2 cores.

Sharding: data-parallel on batch (2 groups of 4 cores); within a group,
tensor-parallel on query heads (4 heads/core). Each core computes its KV head's
k/v projection locally (duplicated across the 2 cores sharing a KV head).
o_proj is row-parallel; the all-reduce over the 4 cores of a group happens on
the host after gather.

Single fused pipeline, everything SBUF-resident (no DRAM round trips):
k/v projections (k first, 8 PSUM banks, matching the xt DMA ramp), then per
q-head {gate proj, q proj + rope, attention, gating}, then o_proj as a tail
block reading the gated values straight from SBUF. All matmul operands are
bf16 (1 cycle/row on the PE like fp32r, but 2x cheaper LDWEIGHTS that fully
hide under the matmul stream, half the SBUF/HBM footprint). PSUM
accumulation is fp32 throughout.

Causal structure: 512-wide q columns; the 4 diagonal k-chunks of each column
only compute the valid suffix (N = 512-128j), with a single [128,128]
triangular mask applied to the first 128 columns of each diagonal chunk.
Score matmuls are emitted up to two k-chunks ahead of the softmax/av chain
so the exp activation latency hides under PE work.

Scalar-engine discipline (the in-order PE otherwise stalls on it at head
boundaries): the scalar engine only ever runs EXP and COPY (one activation
table load total. The gate is stored as exp(-garg) and sigmoid is folded
into the gating denominator, gat = av / (sm*e + sm), evaluated with the
fast approximate reciprocal on DVE. Each head's first projection chunk is
emitted inside the previous head's attention tail (tail_fill), as is the
first o_proj block, so the PE self-fills while scalar drains exps.
"""

import sys

if "/opt/trn_rl_repo" not in sys.path:
    sys.path.insert(0, "/opt/trn_rl_repo")

import ml_dtypes
import numpy as np

import concourse.bass as bass
import concourse.tile as tile
from concourse import bacc, mybir
from concourse.bass_utils import run_bass_kernel_spmd

F32 = mybir.dt.float32
BF16 = mybir.dt.bfloat16
AF = mybir.ActivationFunctionType

S = 2048          # tokens per batch element
E = 2048          # model dim
D = 256           # head dim
RD = 64           # rope dims
NHC = 4           # q heads per core
HD = NHC * D      # per-core head dims (1024)
ECH = E // 128    # 16 contraction chunks
QCH = HD // 128   # 8 per-core q/g/o d-chunks
TT = 4            # 512-wide token tiles
NKC = S // 128    # 16 k chunks
NQC = S // 128    # 16 q chunks (oproj)


def _body(tc, d):
    nc = tc.nc
    ts = bass.ts

    from contextlib import ExitStack

    stack = ExitStack()

    pm = stack.enter_context(tc.tile_pool(name="main", bufs=1))
    kt = pm.tile([128, 2, S], BF16, tag="kt")
    vt = pm.tile([128, NKC, D], BF16, tag="vt")
    gat = pm.tile([128, QCH, S], BF16, tag="gat")
    ones = pm.tile([128, 128], BF16, tag="ones")
    nc.gpsimd.dma_start(ones[:], d["ones"].ap())
    tri = pm.tile([128, 128], BF16, tag="tri")
    nc.gpsimd.dma_start(tri[:], d["tri"].ap())
    rotm = pm.tile([RD, RD], BF16, tag="rotm")
    nc.gpsimd.dma_start(rotm[:], d["rotm"].ap())
    cos_t = pm.tile([RD, S], BF16, tag="cos")
    nc.gpsimd.dma_start(cos_t[:], d["cost"].ap())
    sin_t = pm.tile([RD, S], BF16, tag="sin")
    nc.gpsimd.dma_start(sin_t[:], d["sint"].ap())

    p_qg = pm
    p_exp = pm
    p_gt2 = pm
    p_rtmp = pm
    psum = stack.enter_context(tc.tile_pool(name="psum", bufs=8, space="PSUM"))

    def rope(dst, t):
        # dst: [128, 512] SBUF slice whose partitions 0:RD hold rope dims.
        # rot = R @ x via PE, then dst[0:RD] = x*cos + rot*sin.
        rp = psum.tile([RD, 512], F32, tag="ps")
        nc.tensor.matmul(rp[:], rotm[:], dst[0:RD, :], start=True, stop=True)
        tmp = p_rtmp.tile([RD, 512], F32, tag="rt", bufs=2)
        nc.vector.tensor_mul(tmp[:], dst[0:RD, :], cos_t[:, ts(t, 512)])
        nc.vector.tensor_mul(dst[0:RD, :], rp[:], sin_t[:, ts(t, 512)])
        nc.vector.tensor_add(dst[0:RD, :], dst[0:RD, :], tmp[:])

    def attention(h, q_h, g_h, tail_fill=None):
        tasks = [(qq, kk) for qq in range(TT) for kk in range(4 * qq + 4)]
        sps = {}
        col = {}

        def emit_sp(qq, kk):
            j = kk - 4 * qq
            off = 128 * j if j > 0 else 0
            w = 512 - off
            spt = psum.tile([128, 512], F32, tag="ps")
            sp = spt[:, 0:w]
            base = 512 * qq + off
            nc.tensor.matmul(
                sp, kt[:, 0, ts(kk, 128)], q_h[:, 0, base : 512 * qq + 512],
                start=True, stop=False,
            )
            nc.tensor.matmul(
                sp, kt[:, 1, ts(kk, 128)], q_h[:, 1, base : 512 * qq + 512],
                start=False, stop=True,
            )
            sps[(qq, kk)] = (sp, off, w)

        for i0 in range(3):
            emit_sp(*tasks[i0])
        emitted = 3
        for idx, (qq, kk) in enumerate(tasks):
            # keep the score pipeline 2 deep mid-column (hides exp latency on
            # the narrow diagonal chunks); drop to 1 at column boundaries so
            # PSUM never exceeds 8 live banks.
            while emitted < len(tasks) and (
                emitted == idx + 1
                or (emitted == idx + 2 and tasks[emitted][0] == qq and kk > 0)
            ):
                emit_sp(*tasks[emitted])
                emitted += 1
            if idx + 1 == len(tasks) and tail_fill is not None:
                # next head's first projection chunk: PE fills the pipe while
                # the scalar engine drains the last exp activations.
                tail_fill()
            sp, off, w = sps.pop((qq, kk))
            ext = p_exp.tile([128, 512], BF16, tag="ex", bufs=6)
            ex = ext[:, 0:w]
            nc.scalar.activation(ex, sp, AF.Exp, scale=0.0625)
            if kk - 4 * qq >= 0:
                nc.vector.tensor_mul(ex[:, 0:128], ex[:, 0:128], tri[:])
            if kk == 0:
                col[qq] = (
                    psum.tile([128, 512], F32, tag="ps", name="av0"),
                    psum.tile([128, 512], F32, tag="ps", name="av1"),
                    psum.tile([128, 512], F32, tag="ps", name="sm"),
                )
            av0, av1, sm = col[qq]
            st, en = (kk == 0), (kk == 4 * qq + 3)
            nc.tensor.matmul(av0[:, off:512], vt[:, kk, 0:128], ex, start=st, stop=en)
            nc.tensor.matmul(av1[:, off:512], vt[:, kk, 128:256], ex, start=st, stop=en)
            nc.tensor.matmul(sm[:, off:512], ones[:], ex, start=st, stop=en)
            if en:
                # g_h holds exp(-garg); fold the gate and softmax
                # denominators: gat = av / (sm*(1+e^-g)) = av / (sm*e + sm)
                # via one fast approximate reciprocal (~18 bits) per half.
                for c, avc in ((0, av0), (1, av1)):
                    den = p_gt2.tile([128, 512], F32, tag="gtmp", bufs=2)
                    nc.vector.tensor_mul(den[:], sm[:], g_h[:, c, ts(qq, 512)])
                    nc.vector.tensor_add(den[:], den[:], sm[:])
                    rec = p_gt2.tile([128, 512], F32, tag="rec", bufs=2)
                    nc.vector.reciprocal_approx_fast(rec[:], den[:])
                    with nc.allow_low_precision(reason="bf16 gated-attn by design"):
                        nc.vector.tensor_mul(
                            gat[:, 2 * h + c, ts(qq, 512)], avc[:], rec[:]
                        )
                del col[qq]

    # ---------------- Phase A: projections + attention (xt resident) --------
    last_qg = None
    with (
        tc.tile_pool(name="xt", bufs=1) as p_xt,
        tc.tile_pool(name="w", bufs=3) as p_w,
        tc.tile_pool(name="wv", bufs=1) as p_wv,
    ):
        # DMA schedule: the PE's first work is v(ec=0) then k(ec=0), so land
        # xt[ec0] (scalar queue, ahead of everything), then wv/wk quarters
        # interleaved with even xt chunks on sync; odd xt chunks on scalar.
        # All three queues are hardware-DGE.  wq prefetches ride on sync.
        wv_t = p_wv.tile([128, ECH, D], BF16, tag="wv")
        wkt = p_wv.tile([128, 2, ECH, 128], BF16, tag="wk")
        xt = p_xt.tile([128, ECH, S], BF16, tag="xt")

        nc.scalar.dma_start(xt[:, 0, :], d["xt"].ap()[:, 0, :])
        for wh in range(4):
            nc.sync.dma_start(
                wkt[:, 0, ts(wh, 4), :], d["wk"].ap()[0][:, ts(wh, 4), :]
            )
            nc.sync.dma_start(
                wkt[:, 1, ts(wh, 4), :], d["wk"].ap()[1][:, ts(wh, 4), :]
            )
            nc.sync.dma_start(xt[:, 2 + 2 * wh, :], d["xt"].ap()[:, 2 + 2 * wh, :])
        for ec in (1, 3, 5, 7, 9, 11, 13, 15):
            nc.scalar.dma_start(xt[:, ec, :], d["xt"].ap()[:, ec, :])
        for ec in (10, 12, 14):
            nc.sync.dma_start(xt[:, ec, :], d["xt"].ap()[:, ec, :])
        # wv lands after the k-pass xt chunks: v's passes only start ~27us in
        for wh in range(4):
            nc.sync.dma_start(
                wv_t[:, ts(wh, 4), :], d["wv"].ap()[:, ts(wh, 4), :]
            )

        # both k chunks first, ec-outer (8 PSUM banks, ~1.7us of PE burn per
        # 512KB xt chunk): matches the DMA arrival rate during the ramp.
        # PSUM accumulation is commutative, so consume ec chunks in their
        # two-queue arrival order (scalar: 0,1,3,..; sync: 2,4,..) instead of
        # numerically — the in-order PE never blocks on a late early chunk.
        ECS = [0, 2, 1, 4, 3, 6, 5, 8, 7, 10, 9, 12, 11, 14, 13, 15]
        kss = []
        for p in range(2):
            for t in range(TT):
                pk = psum.tile([128, 512], F32, tag="ps", name="pk")
                kss.append(pk)
        for pos, ec in enumerate(ECS):
            for p in range(2):
                for t in range(TT):
                    nc.tensor.matmul(
                        kss[4 * p + t][:],
                        wkt[:, p, ec, :],
                        xt[:, ec, ts(t, 512)],
                        start=(pos == 0),
                        stop=(pos == ECH - 1),
                    )
        for p in range(2):
            for t in range(TT):
                nc.scalar.copy(kt[:, p, ts(t, 512)], kss[4 * p + t][:])

        # v token-chunks, 8 banks at a time
        for wave in range(2):
            pss = []
            for i in range(8):
                pv = psum.tile([128, D], F32, tag="ps", name="pv")
                pss.append(pv)
            for ec in range(ECH):
                for i in range(8):
                    nc.tensor.matmul(
                        pss[i][:],
                        xt[:, ec, ts(8 * wave + i, 128)],
                        wv_t[:, ec, :],
                        start=(ec == 0),
                        stop=(ec == ECH - 1),
                    )
            for i in range(8):
                nc.scalar.copy(vt[:, 8 * wave + i, :], pss[i][:])

        # k rope here: its PSUM->SBUF copies completed during the v tail, so
        # the rp matmuls never wait on the scalar engine.
        for t in range(TT):
            rope(kt[:, 0, ts(t, 512)], t)

        def fetch_w(w_ap):
            wt = p_w.tile([128, ECH, 128], BF16, tag="w", name="wt")
            nc.sync.dma_start(wt[:], w_ap)
            return wt

        def proj_mm(wt):
            pss = []
            for t in range(TT):
                pt = psum.tile([128, 512], F32, tag="ps")
                pss.append(pt)
            for ec in range(ECH):
                for t in range(TT):
                    nc.tensor.matmul(
                        pss[t][:],
                        wt[:, ec, :],
                        xt[:, ec, ts(t, 512)],
                        start=(ec == 0),
                        stop=(ec == ECH - 1),
                    )
            return pss

        def proj_fin(pss, dst, dst_idx, kind):
            for t in range(TT):
                dslice = dst[:, dst_idx, ts(t, 512)]
                if kind == "g":
                    # store exp(-garg): the scalar engine stays on the EXP
                    # table (no 1.3us reloads); sigmoid is folded into the
                    # gating denominator as sm*(1+e) = sm*e + sm.
                    nc.scalar.activation(dslice, pss[t][:], AF.Exp, scale=-1.0)
                elif kind == "q" and dst_idx == 1:
                    # q chunk 1 finishes right before attention: route its
                    # copies to the vector engine so the scalar queue is
                    # already clear for attention's first exp.
                    with nc.allow_low_precision(reason="bf16 q storage by design"):
                        nc.vector.tensor_scalar_add(dslice, pss[t][:], 0.0)
                else:
                    nc.scalar.copy(dslice, pss[t][:])

        def proj_chunk(w_ap, dst, dst_idx, kind, wt=None):
            if wt is None:
                wt = fetch_w(w_ap)
            proj_fin(proj_mm(wt), dst, dst_idx, kind)

        # Each head's first weight chunk is prefetched on the sync queue and
        # its 64 projection matmuls are emitted inside the previous head's
        # attention tail (tail_fill), so the PE never idles at head
        # boundaries while the scalar engine drains exp activations.
        def make_qg():
            q_h = p_qg.tile([128, 2, S], BF16, tag="qh", bufs=2)
            g_h = p_qg.tile([128, 2, S], BF16, tag="gh", bufs=2)
            return q_h, g_h

        # Per-head order g0,g1,q0,rope,q1: the gate exps land early on the
        # scalar queue, so it is empty when attention's first exp arrives.
        cur_qg = make_qg()
        pending = [proj_mm(fetch_w(d["wg"].ap()[0]))]
        for h in range(NHC):
            q_h, g_h = cur_qg
            proj_fin(pending.pop(), g_h, 0, "g")
            proj_chunk(d["wg"].ap()[2 * h + 1], g_h, 1, "g")
            proj_chunk(d["wq"].ap()[2 * h], q_h, 0, "q")
            for t in range(TT):
                rope(q_h[:, 0, ts(t, 512)], t)
            proj_chunk(d["wq"].ap()[2 * h + 1], q_h, 1, "q")
            if h < NHC - 1:
                cur_qg = make_qg()
                pre = fetch_w(d["wg"].ap()[2 * h + 2])

                def tail_fill(wt=pre):
                    pending.append(proj_mm(wt))

                attention(h, q_h, g_h, tail_fill=tail_fill)
            else:
                last_qg = (q_h, g_h)

    # ---------------- Phase B: last head's attention + o_proj ---------------
    with (
        tc.tile_pool(name="wo", bufs=1) as p_wo,
        tc.tile_pool(name="ob", bufs=4) as p_ob,
    ):
        wo_t = p_wo.tile([128, QCH, E], BF16, tag="wo")
        for i in range(4):
            nc.sync.dma_start(
                wo_t[:, 2 * i : 2 * i + 2, :], d["wo"].ap()[:, 2 * i : 2 * i + 2, :]
            )

        def oproj_mm(qc):
            ops = []
            for et in range(4):
                op = psum.tile([128, 512], F32, tag="ps", name="op")
                ops.append(op)
            for hc in range(QCH):
                gd = gat[:, hc, ts(qc, 128)]
                for et in range(4):
                    nc.tensor.matmul(
                        ops[et][:],
                        gd,
                        wo_t[:, hc, ts(et, 512)],
                        start=(hc == 0),
                        stop=(hc == QCH - 1),
                    )
            return ops

        pend_o = []

        def tail_fill3():
            pend_o.append(oproj_mm(0))

        attention(NHC - 1, *last_qg, tail_fill=tail_fill3)

        for qc in range(NQC):
            ops = pend_o.pop() if qc == 0 else oproj_mm(qc)
            ob = p_ob.tile([128, E], BF16, tag="ob")
            for et in range(4):
                nc.scalar.copy(ob[:, ts(et, 512)], ops[et][:])
                nc.sync.dma_start(
                    d["out"].ap()[qc][:, ts(et, 512)], ob[:, ts(et, 512)]
                )

    stack.close()


def build_nc():
    nc = bacc.Bacc("TRN2", target_bir_lowering=False, debug=False)
    d = {}
    d["xt"] = nc.dram_tensor("xt", [128, ECH, S], BF16, kind="ExternalInput")
    d["wq"] = nc.dram_tensor("wq", [QCH, 128, ECH, 128], BF16, kind="ExternalInput")
    d["wg"] = nc.dram_tensor("wg", [QCH, 128, ECH, 128], BF16, kind="ExternalInput")
    d["wk"] = nc.dram_tensor("wk", [2, 128, ECH, 128], BF16, kind="ExternalInput")
    d["wv"] = nc.dram_tensor("wv", [128, ECH, D], BF16, kind="ExternalInput")
    d["wo"] = nc.dram_tensor("wo", [128, QCH, E], BF16, kind="ExternalInput")
    d["cost"] = nc.dram_tensor("cost", [RD, S], BF16, kind="ExternalInput")
    d["sint"] = nc.dram_tensor("sint", [RD, S], BF16, kind="ExternalInput")
    d["tri"] = nc.dram_tensor("tri", [128, 128], BF16, kind="ExternalInput")
    d["rotm"] = nc.dram_tensor("rotm", [RD, RD], BF16, kind="ExternalInput")
    d["ones"] = nc.dram_tensor("ones", [128, 128], BF16, kind="ExternalInput")
    d["out"] = nc.dram_tensor("out", [NQC, 128, E], BF16, kind="ExternalOutput")
    with tile.TileContext(nc) as tc:
        _body(tc, d)
    nc.compile()
    return nc


_NC_CACHE = None


def _get_nc():
    global _NC_CACHE
    if _NC_CACHE is None:
        _NC_CACHE = build_nc()
    return _NC_CACHE


def _rope_tables():
    inv = 1.0 / (10000.0 ** (np.arange(0, RD, 2, dtype=np.float32) / np.float32(RD)))
    t = np.arange(S, dtype=np.float32)
    freqs = np.outer(t, inv).astype(np.float32)          # [S, RD/2]
    emb = np.concatenate([freqs, freqs], axis=1)         # [S, RD]
    return (
        np.ascontiguousarray(np.cos(emb).T).astype(ml_dtypes.bfloat16),
        np.ascontiguousarray(np.sin(emb).T).astype(ml_dtypes.bfloat16),
    )


def _rotm():
    r = np.zeros((RD, RD), dtype=np.float32)  # r[j, d] = R[d, j], rot = R @ x
    half = RD // 2
    for dd in range(half):
        r[dd + half, dd] = -1.0
    for dd in range(half, RD):
        r[dd - half, dd] = 1.0
    return r.astype(ml_dtypes.bfloat16)


def _tri():
    k = np.arange(128)[:, None]
    q = np.arange(128)[None, :]
    return (k <= q).astype(ml_dtypes.bfloat16)


def _prep_in_maps(hidden_states, Wq, Wk, Wv, Wg, Wo):
    cosT, sinT = _rope_tables()
    tri = _tri()
    rotm = _rotm()
    ones = np.ones((128, 128), dtype=ml_dtypes.bfloat16)
    maps = []
    for c in range(8):
        b, t = c // 4, c % 4
        hq0, kvh = 4 * t, (t // 2)
        cols = slice(hq0 * D, (hq0 + NHC) * D)
        kcols = slice(kvh * D, (kvh + 1) * D)
        x = hidden_states[b]  # [S, E]
        m = {
            "xt": np.ascontiguousarray(
                x.T.reshape(ECH, 128, S).transpose(1, 0, 2)
            ).astype(ml_dtypes.bfloat16),
            "wq": np.ascontiguousarray(
                Wq[:, cols].reshape(ECH, 128, QCH, 128).transpose(2, 1, 0, 3)
            ).astype(ml_dtypes.bfloat16),
            "wg": np.ascontiguousarray(
                Wg[:, cols].reshape(ECH, 128, QCH, 128).transpose(2, 1, 0, 3)
            ).astype(ml_dtypes.bfloat16),
            "wk": np.ascontiguousarray(
                Wk[:, kcols].reshape(ECH, 128, 2, 128).transpose(2, 1, 0, 3)
            ).astype(ml_dtypes.bfloat16),
            "wv": np.ascontiguousarray(
                Wv[:, kcols].reshape(ECH, 128, D).transpose(1, 0, 2)
            ).astype(ml_dtypes.bfloat16),
            "wo": np.ascontiguousarray(
                Wo[cols, :].reshape(QCH, 128, E).transpose(1, 0, 2)
            ).astype(ml_dtypes.bfloat16),
            "cost": cosT,
            "sint": sinT,
            "tri": tri,
            "rotm": rotm,
            "ones": ones,
        }
        maps.append(m)
    return maps


def _run(inputs, trace=False, trace_cores=None, tmpdir=None):
    nc = _get_nc()
    in_maps = _prep_in_maps(**inputs)
    kw = {}
    if trace:
        kw = dict(trace=True, trace_cores=trace_cores, tmpdir=tmpdir)
    res = run_bass_kernel_spmd(nc, in_maps, list(range(8)), **kw)
    outs = [
        np.asarray(res.results[c]["out"], dtype=np.float32).reshape(S, E)
        for c in range(8)
    ]
    full = np.stack(
        [
            outs[0] + outs[1] + outs[2] + outs[3],
            outs[4] + outs[5] + outs[6] + outs[7],
        ]
    ).astype(np.float32)
    return full, res


def kernel(hidden_states, Wq, Wk, Wv, Wg, Wo):
    full, _ = _run(
        dict(hidden_states=np.asarray(hidden_states, dtype=np.float32),
             Wq=np.asarray(Wq, dtype=np.float32),
             Wk=np.asarray(Wk, dtype=np.float32),
             Wv=np.asarray(Wv, dtype=np.float32),
             Wg=np.asarray(Wg, dtype=np.float32),
             Wo=np.asarray(Wo, dtype=np.float32))
    )
    return full


if __name__ == "__main__":
    nc = build_nc()
    print("build OK")


# revision 57
# speedup vs baseline: 1.0042x; 1.0042x over previous
"""Gated GQA attention block (B=2,S=2048,E=2048,H=16,HKV=2,D=256,RD=64) on 8 TRN2 cores.

Sharding: data-parallel on batch (2 groups of 4 cores); within a group,
tensor-parallel on query heads (4 heads/core). Each core computes its KV head's
k/v projection locally (duplicated across the 2 cores sharing a KV head).
o_proj is row-parallel; the all-reduce over the 4 cores of a group happens on
the host after gather.

Single fused pipeline, everything SBUF-resident (no DRAM round trips):
k/v projections (k first, 8 PSUM banks, matching the xt DMA ramp), then per
q-head {gate proj, q proj + rope, attention, gating}, then o_proj as a tail
block reading the gated values straight from SBUF. All matmul operands are
bf16 (1 cycle/row on the PE like fp32r, but 2x cheaper LDWEIGHTS that fully
hide under the matmul stream, half the SBUF/HBM footprint). PSUM
accumulation is fp32 throughout.

Causal structure: 512-wide q columns; the 4 diagonal k-chunks of each column
only compute the valid suffix (N = 512-128j), with a single [128,128]
triangular mask applied to the first 128 columns of each diagonal chunk.
Score matmuls are emitted up to two k-chunks ahead of the softmax/av chain
so the exp activation latency hides under PE work.

Scalar-engine discipline (the in-order PE otherwise stalls on it at head
boundaries): the scalar engine only ever runs EXP and COPY (one activation
table load total. The gate is stored as exp(-garg) and sigmoid is folded
into the gating denominator, gat = av / (sm*e + sm), evaluated with the
fast approximate reciprocal on DVE. Each head's first projection chunk is
emitted inside the previous head's attention tail (tail_fill), as is the
first o_proj block, so the PE self-fills while scalar drains exps.
"""

import sys

if "/opt/trn_rl_repo" not in sys.path:
    sys.path.insert(0, "/opt/trn_rl_repo")

import ml_dtypes
import numpy as np

import concourse.bass as bass
import concourse.tile as tile
from concourse import bacc, mybir
from concourse.bass_utils import run_bass_kernel_spmd

F32 = mybir.dt.float32
BF16 = mybir.dt.bfloat16
AF = mybir.ActivationFunctionType

S = 2048          # tokens per batch element
E = 2048          # model dim
D = 256           # head dim
RD = 64           # rope dims
NHC = 4           # q heads per core
HD = NHC * D      # per-core head dims (1024)
ECH = E // 128    # 16 contraction chunks
QCH = HD // 128   # 8 per-core q/g/o d-chunks
TT = 4            # 512-wide token tiles
NKC = S // 128    # 16 k chunks
NQC = S // 128    # 16 q chunks (oproj)


def _body(tc, d):
    nc = tc.nc
    ts = bass.ts

    from contextlib import ExitStack

    stack = ExitStack()

    pm = stack.enter_context(tc.tile_pool(name="main", bufs=1))
    kt = pm.tile([128, 2, S], BF16, tag="kt")
    vt = pm.tile([128, NKC, D], BF16, tag="vt")
    gat = pm.tile([128, QCH, S], BF16, tag="gat")
    ones = pm.tile([128, 128], BF16, tag="ones")
    nc.gpsimd.dma_start(ones[:], d["ones"].ap())
    tri = pm.tile([128, 128], BF16, tag="tri")
    nc.gpsimd.dma_start(tri[:], d["tri"].ap())
    rotm = pm.tile([RD, RD], BF16, tag="rotm")
    nc.gpsimd.dma_start(rotm[:], d["rotm"].ap())
    cos_t = pm.tile([RD, S], BF16, tag="cos")
    nc.gpsimd.dma_start(cos_t[:], d["cost"].ap())
    sin_t = pm.tile([RD, S], BF16, tag="sin")
    nc.gpsimd.dma_start(sin_t[:], d["sint"].ap())

    p_qg = pm
    p_exp = pm
    p_gt2 = pm
    p_rtmp = pm
    psum = stack.enter_context(tc.tile_pool(name="psum", bufs=8, space="PSUM"))

    def rope(dst, t):
        # dst: [128, 512] SBUF slice whose partitions 0:RD hold rope dims.
        # rot = R @ x via PE, then dst[0:RD] = x*cos + rot*sin.
        rp = psum.tile([RD, 512], F32, tag="ps")
        nc.tensor.matmul(rp[:], rotm[:], dst[0:RD, :], start=True, stop=True)
        tmp = p_rtmp.tile([RD, 512], F32, tag="rt", bufs=2)
        nc.vector.tensor_mul(tmp[:], dst[0:RD, :], cos_t[:, ts(t, 512)])
        nc.vector.tensor_mul(dst[0:RD, :], rp[:], sin_t[:, ts(t, 512)])
        nc.vector.tensor_add(dst[0:RD, :], dst[0:RD, :], tmp[:])

    def attention(h, q_h, g_h, tail_fill=None):
        tasks = [(qq, kk) for qq in range(TT) for kk in range(4 * qq + 4)]
        sps = {}
        col = {}

        def emit_sp(qq, kk):
            j = kk - 4 * qq
            off = 128 * j if j > 0 else 0
            w = 512 - off
            spt = psum.tile([128, 512], F32, tag="ps")
            sp = spt[:, 0:w]
            base = 512 * qq + off
            nc.tensor.matmul(
                sp, kt[:, 0, ts(kk, 128)], q_h[:, 0, base : 512 * qq + 512],
                start=True, stop=False,
            )
            nc.tensor.matmul(
                sp, kt[:, 1, ts(kk, 128)], q_h[:, 1, base : 512 * qq + 512],
                start=False, stop=True,
            )
            sps[(qq, kk)] = (sp, off, w)

        for i0 in range(3):
            emit_sp(*tasks[i0])
        emitted = 3
        for idx, (qq, kk) in enumerate(tasks):
            # keep the score pipeline 2 deep mid-column (hides exp latency on
            # the narrow diagonal chunks); drop to 1 at column boundaries so
            # PSUM never exceeds 8 live banks.
            while emitted < len(tasks) and (
                emitted == idx + 1
                or (emitted == idx + 2 and tasks[emitted][0] == qq and kk > 0)
            ):
                emit_sp(*tasks[emitted])
                emitted += 1
            if idx + 1 == len(tasks) and tail_fill is not None:
                # next head's first projection chunk: PE fills the pipe while
                # the scalar engine drains the last exp activations.
                tail_fill()
            sp, off, w = sps.pop((qq, kk))
            ext = p_exp.tile([128, 512], BF16, tag="ex", bufs=6)
            ex = ext[:, 0:w]
            nc.scalar.activation(ex, sp, AF.Exp, scale=0.0625)
            if kk - 4 * qq >= 0:
                nc.vector.tensor_mul(ex[:, 0:128], ex[:, 0:128], tri[:])
            if kk == 0:
                col[qq] = (
                    psum.tile([128, 512], F32, tag="ps", name="av0"),
                    psum.tile([128, 512], F32, tag="ps", name="av1"),
                    psum.tile([128, 512], F32, tag="ps", name="sm"),
                )
            av0, av1, sm = col[qq]
            st, en = (kk == 0), (kk == 4 * qq + 3)
            nc.tensor.matmul(av0[:, off:512], vt[:, kk, 0:128], ex, start=st, stop=en)
            nc.tensor.matmul(av1[:, off:512], vt[:, kk, 128:256], ex, start=st, stop=en)
            nc.tensor.matmul(sm[:, off:512], ones[:], ex, start=st, stop=en)
            if en:
                # g_h holds exp(-garg); fold the gate and softmax
                # denominators: gat = av / (sm*(1+e^-g)) = av / (sm*e + sm)
                # via one fast approximate reciprocal (~18 bits) per half.
                for c, avc in ((0, av0), (1, av1)):
                    den = p_gt2.tile([128, 512], F32, tag="gtmp", bufs=2)
                    nc.vector.tensor_mul(den[:], sm[:], g_h[:, c, ts(qq, 512)])
                    nc.vector.tensor_add(den[:], den[:], sm[:])
                    rec = p_gt2.tile([128, 512], F32, tag="rec", bufs=2)
                    nc.vector.reciprocal_approx_fast(rec[:], den[:])
                    with nc.allow_low_precision(reason="bf16 gated-attn by design"):
                        nc.vector.tensor_mul(
                            gat[:, 2 * h + c, ts(qq, 512)], avc[:], rec[:]
                        )
                del col[qq]

    # ---------------- Phase A: projections + attention (xt resident) --------
    last_qg = None
    with (
        tc.tile_pool(name="xt", bufs=1) as p_xt,
        tc.tile_pool(name="w", bufs=3) as p_w,
        tc.tile_pool(name="wv", bufs=1) as p_wv,
    ):
        # DMA schedule: the PE's first work is v(ec=0) then k(ec=0), so land
        # xt[ec0] (scalar queue, ahead of everything), then wv/wk quarters
        # interleaved with even xt chunks on sync; odd xt chunks on scalar.
        # All three queues are hardware-DGE.  wq prefetches ride on sync.
        wv_t = p_wv.tile([128, ECH, D], BF16, tag="wv")
        wkt = p_wv.tile([128, 2, ECH, 128], BF16, tag="wk")
        xt = p_xt.tile([128, ECH, S], BF16, tag="xt")

        # first chunk in 512-column quarters so ec0's matmuls start with the
        # first 128KB instead of waiting for the whole 512KB transfer
        for t in range(TT):
            nc.scalar.dma_start(
                xt[:, 0, ts(t, 512)], d["xt"].ap()[:, 0, ts(t, 512)]
            )
        for wh in range(4):
            nc.sync.dma_start(
                wkt[:, 0, ts(wh, 4), :], d["wk"].ap()[0][:, ts(wh, 4), :]
            )
            nc.sync.dma_start(
                wkt[:, 1, ts(wh, 4), :], d["wk"].ap()[1][:, ts(wh, 4), :]
            )
            nc.sync.dma_start(xt[:, 2 + 2 * wh, :], d["xt"].ap()[:, 2 + 2 * wh, :])
        for ec in (1, 3, 5, 7, 9, 11, 13, 15):
            nc.scalar.dma_start(xt[:, ec, :], d["xt"].ap()[:, ec, :])
        for ec in (10, 12, 14):
            nc.sync.dma_start(xt[:, ec, :], d["xt"].ap()[:, ec, :])
        # wv lands after the k-pass xt chunks: v's passes only start ~27us in
        for wh in range(4):
            nc.sync.dma_start(
                wv_t[:, ts(wh, 4), :], d["wv"].ap()[:, ts(wh, 4), :]
            )

        # both k chunks first, ec-outer (8 PSUM banks, ~1.7us of PE burn per
        # 512KB xt chunk): matches the DMA arrival rate during the ramp.
        # PSUM accumulation is commutative, so consume ec chunks in their
        # two-queue arrival order (scalar: 0,1,3,..; sync: 2,4,..) instead of
        # numerically — the in-order PE never blocks on a late early chunk.
        ECS = [0, 2, 1, 4, 3, 6, 5, 8, 7, 10, 9, 12, 11, 14, 13, 15]
        kss = []
        for p in range(2):
            for t in range(TT):
                pk = psum.tile([128, 512], F32, tag="ps", name="pk")
                kss.append(pk)
        for pos, ec in enumerate(ECS):
            # t-major: each arriving 512-column quarter of an xt chunk
            # unlocks both k-chunk matmuls immediately
            for t in range(TT):
                for p in range(2):
                    nc.tensor.matmul(
                        kss[4 * p + t][:],
                        wkt[:, p, ec, :],
                        xt[:, ec, ts(t, 512)],
                        start=(pos == 0),
                        stop=(pos == ECH - 1),
                    )
        for p in range(2):
            for t in range(TT):
                nc.scalar.copy(kt[:, p, ts(t, 512)], kss[4 * p + t][:])

        # v token-chunks, 8 banks at a time
        for wave in range(2):
            pss = []
            for i in range(8):
                pv = psum.tile([128, D], F32, tag="ps", name="pv")
                pss.append(pv)
            for ec in range(ECH):
                for i in range(8):
                    nc.tensor.matmul(
                        pss[i][:],
                        xt[:, ec, ts(8 * wave + i, 128)],
                        wv_t[:, ec, :],
                        start=(ec == 0),
                        stop=(ec == ECH - 1),
                    )
            for i in range(8):
                nc.scalar.copy(vt[:, 8 * wave + i, :], pss[i][:])

        # k rope here: its PSUM->SBUF copies completed during the v tail, so
        # the rp matmuls never wait on the scalar engine.
        for t in range(TT):
            rope(kt[:, 0, ts(t, 512)], t)

        def fetch_w(w_ap):
            wt = p_w.tile([128, ECH, 128], BF16, tag="w", name="wt")
            nc.sync.dma_start(wt[:], w_ap)
            return wt

        def proj_mm(wt):
            pss = []
            for t in range(TT):
                pt = psum.tile([128, 512], F32, tag="ps")
                pss.append(pt)
            for ec in range(ECH):
                for t in range(TT):
                    nc.tensor.matmul(
                        pss[t][:],
                        wt[:, ec, :],
                        xt[:, ec, ts(t, 512)],
                        start=(ec == 0),
                        stop=(ec == ECH - 1),
                    )
            return pss

        def proj_fin(pss, dst, dst_idx, kind):
            for t in range(TT):
                dslice = dst[:, dst_idx, ts(t, 512)]
                if kind == "g":
                    # store exp(-garg): the scalar engine stays on the EXP
                    # table (no 1.3us reloads); sigmoid is folded into the
                    # gating denominator as sm*(1+e) = sm*e + sm.
                    nc.scalar.activation(dslice, pss[t][:], AF.Exp, scale=-1.0)
                elif kind == "q" and dst_idx == 1:
                    # q chunk 1 finishes right before attention: route its
                    # copies to the vector engine so the scalar queue is
                    # already clear for attention's first exp.
                    with nc.allow_low_precision(reason="bf16 q storage by design"):
                        nc.vector.tensor_scalar_add(dslice, pss[t][:], 0.0)
                else:
                    nc.scalar.copy(dslice, pss[t][:])

        def proj_chunk(w_ap, dst, dst_idx, kind, wt=None):
            if wt is None:
                wt = fetch_w(w_ap)
            proj_fin(proj_mm(wt), dst, dst_idx, kind)

        # Each head's first weight chunk is prefetched on the sync queue and
        # its 64 projection matmuls are emitted inside the previous head's
        # attention tail (tail_fill), so the PE never idles at head
        # boundaries while the scalar engine drains exp activations.
        def make_qg():
            q_h = p_qg.tile([128, 2, S], BF16, tag="qh", bufs=2)
            g_h = p_qg.tile([128, 2, S], BF16, tag="gh", bufs=2)
            return q_h, g_h

        # Per-head order g0,g1,q0,rope,q1: the gate exps land early on the
        # scalar queue, so it is empty when attention's first exp arrives.
        cur_qg = make_qg()
        pending = [proj_mm(fetch_w(d["wg"].ap()[0]))]
        for h in range(NHC):
            q_h, g_h = cur_qg
            proj_fin(pending.pop(), g_h, 0, "g")
            proj_chunk(d["wg"].ap()[2 * h + 1], g_h, 1, "g")
            proj_chunk(d["wq"].ap()[2 * h], q_h, 0, "q")
            for t in range(TT):
                rope(q_h[:, 0, ts(t, 512)], t)
            proj_chunk(d["wq"].ap()[2 * h + 1], q_h, 1, "q")
            if h < NHC - 1:
                cur_qg = make_qg()
                pre = fetch_w(d["wg"].ap()[2 * h + 2])

                def tail_fill(wt=pre):
                    pending.append(proj_mm(wt))

                attention(h, q_h, g_h, tail_fill=tail_fill)
            else:
                last_qg = (q_h, g_h)

    # ---------------- Phase B: last head's attention + o_proj ---------------
    with (
        tc.tile_pool(name="wo", bufs=1) as p_wo,
        tc.tile_pool(name="ob", bufs=4) as p_ob,
    ):
        wo_t = p_wo.tile([128, QCH, E], BF16, tag="wo")
        for i in range(4):
            nc.sync.dma_start(
                wo_t[:, 2 * i : 2 * i + 2, :], d["wo"].ap()[:, 2 * i : 2 * i + 2, :]
            )

        def oproj_mm(qc):
            ops = []
            for et in range(4):
                op = psum.tile([128, 512], F32, tag="ps", name="op")
                ops.append(op)
            for hc in range(QCH):
                gd = gat[:, hc, ts(qc, 128)]
                for et in range(4):
                    nc.tensor.matmul(
                        ops[et][:],
                        gd,
                        wo_t[:, hc, ts(et, 512)],
                        start=(hc == 0),
                        stop=(hc == QCH - 1),
                    )
            return ops

        pend_o = []

        def tail_fill3():
            pend_o.append(oproj_mm(0))

        attention(NHC - 1, *last_qg, tail_fill=tail_fill3)

        for qc in range(NQC):
            ops = pend_o.pop() if qc == 0 else oproj_mm(qc)
            ob = p_ob.tile([128, E], BF16, tag="ob")
            for et in range(4):
                nc.scalar.copy(ob[:, ts(et, 512)], ops[et][:])
                nc.sync.dma_start(
                    d["out"].ap()[qc][:, ts(et, 512)], ob[:, ts(et, 512)]
                )

    stack.close()


def build_nc():
    nc = bacc.Bacc("TRN2", target_bir_lowering=False, debug=False)
    d = {}
    d["xt"] = nc.dram_tensor("xt", [128, ECH, S], BF16, kind="ExternalInput")
    d["wq"] = nc.dram_tensor("wq", [QCH, 128, ECH, 128], BF16, kind="ExternalInput")
    d["wg"] = nc.dram_tensor("wg", [QCH, 128, ECH, 128], BF16, kind="ExternalInput")
    d["wk"] = nc.dram_tensor("wk", [2, 128, ECH, 128], BF16, kind="ExternalInput")
    d["wv"] = nc.dram_tensor("wv", [128, ECH, D], BF16, kind="ExternalInput")
    d["wo"] = nc.dram_tensor("wo", [128, QCH, E], BF16, kind="ExternalInput")
    d["cost"] = nc.dram_tensor("cost", [RD, S], BF16, kind="ExternalInput")
    d["sint"] = nc.dram_tensor("sint", [RD, S], BF16, kind="ExternalInput")
    d["tri"] = nc.dram_tensor("tri", [128, 128], BF16, kind="ExternalInput")
    d["rotm"] = nc.dram_tensor("rotm", [RD, RD], BF16, kind="ExternalInput")
    d["ones"] = nc.dram_tensor("ones", [128, 128], BF16, kind="ExternalInput")
    d["out"] = nc.dram_tensor("out", [NQC, 128, E], BF16, kind="ExternalOutput")
    with tile.TileContext(nc) as tc:
        _body(tc, d)
    nc.compile()
    return nc


_NC_CACHE = None


def _get_nc():
    global _NC_CACHE
    if _NC_CACHE is None:
        _NC_CACHE = build_nc()
    return _NC_CACHE


def _rope_tables():
    inv = 1.0 / (10000.0 ** (np.arange(0, RD, 2, dtype=np.float32) / np.float32(RD)))
    t = np.arange(S, dtype=np.float32)
    freqs = np.outer(t, inv).astype(np.float32)          # [S, RD/2]
    emb = np.concatenate([freqs, freqs], axis=1)         # [S, RD]
    return (
        np.ascontiguousarray(np.cos(emb).T).astype(ml_dtypes.bfloat16),
        np.ascontiguousarray(np.sin(emb).T).astype(ml_dtypes.bfloat16),
    )


def _rotm():
    r = np.zeros((RD, RD), dtype=np.float32)  # r[j, d] = R[d, j], rot = R @ x
    half = RD // 2
    for dd in range(half):
        r[dd + half, dd] = -1.0
    for dd in range(half, RD):
        r[dd - half, dd] = 1.0
    return r.astype(ml_dtypes.bfloat16)


def _tri():
    k = np.arange(128)[:, None]
    q = np.arange(128)[None, :]
    return (k <= q).astype(ml_dtypes.bfloat16)


def _prep_in_maps(hidden_states, Wq, Wk, Wv, Wg, Wo):
    cosT, sinT = _rope_tables()
    tri = _tri()
    rotm = _rotm()
    ones = np.ones((128, 128), dtype=ml_dtypes.bfloat16)
    maps = []
    for c in range(8):
        b, t = c // 4, c % 4
        hq0, kvh = 4 * t, (t // 2)
        cols = slice(hq0 * D, (hq0 + NHC) * D)
        kcols = slice(kvh * D, (kvh + 1) * D)
        x = hidden_states[b]  # [S, E]
        m = {
            "xt": np.ascontiguousarray(
                x.T.reshape(ECH, 128, S).transpose(1, 0, 2)
            ).astype(ml_dtypes.bfloat16),
            "wq": np.ascontiguousarray(
                Wq[:, cols].reshape(ECH, 128, QCH, 128).transpose(2, 1, 0, 3)
            ).astype(ml_dtypes.bfloat16),
            "wg": np.ascontiguousarray(
                Wg[:, cols].reshape(ECH, 128, QCH, 128).transpose(2, 1, 0, 3)
            ).astype(ml_dtypes.bfloat16),
            "wk": np.ascontiguousarray(
                Wk[:, kcols].reshape(ECH, 128, 2, 128).transpose(2, 1, 0, 3)
            ).astype(ml_dtypes.bfloat16),
            "wv": np.ascontiguousarray(
                Wv[:, kcols].reshape(ECH, 128, D).transpose(1, 0, 2)
            ).astype(ml_dtypes.bfloat16),
            "wo": np.ascontiguousarray(
                Wo[cols, :].reshape(QCH, 128, E).transpose(1, 0, 2)
            ).astype(ml_dtypes.bfloat16),
            "cost": cosT,
            "sint": sinT,
            "tri": tri,
            "rotm": rotm,
            "ones": ones,
        }
        maps.append(m)
    return maps


def _run(inputs, trace=False, trace_cores=None, tmpdir=None):
    nc = _get_nc()
    in_maps = _prep_in_maps(**inputs)
    kw = {}
    if trace:
        kw = dict(trace=True, trace_cores=trace_cores, tmpdir=tmpdir)
    res = run_bass_kernel_spmd(nc, in_maps, list(range(8)), **kw)
    outs = [
        np.asarray(res.results[c]["out"], dtype=np.float32).reshape(S, E)
        for c in range(8)
    ]
    full = np.stack(
        [
            outs[0] + outs[1] + outs[2] + outs[3],
            outs[4] + outs[5] + outs[6] + outs[7],
        ]
    ).astype(np.float32)
    return full, res


def kernel(hidden_states, Wq, Wk, Wv, Wg, Wo):
    full, _ = _run(
        dict(hidden_states=np.asarray(hidden_states, dtype=np.float32),
             Wq=np.asarray(Wq, dtype=np.float32),
             Wk=np.asarray(Wk, dtype=np.float32),
             Wv=np.asarray(Wv, dtype=np.float32),
             Wg=np.asarray(Wg, dtype=np.float32),
             Wo=np.asarray(Wo, dtype=np.float32))
    )
    return full


if __name__ == "__main__":
    nc = build_nc()
    print("build OK")


# revision 61
# speedup vs baseline: 1.0115x; 1.0072x over previous
"""Gated GQA attention block (B=2,S=2048,E=2048,H=16,HKV=2,D=256,RD=64) on 8 TRN2 cores.

Sharding: data-parallel on batch (2 groups of 4 cores); within a group,
tensor-parallel on query heads (4 heads/core). Each core computes its KV head's
k/v projection locally (duplicated across the 2 cores sharing a KV head).
o_proj is row-parallel; the all-reduce over the 4 cores of a group happens on
the host after gather.

Single fused pipeline, everything SBUF-resident (no DRAM round trips):
k/v projections (k first, 8 PSUM banks, matching the xt DMA ramp), then per
q-head {gate proj, q proj + rope, attention, gating}, then o_proj as a tail
block reading the gated values straight from SBUF. All matmul operands are
bf16 (1 cycle/row on the PE like fp32r, but 2x cheaper LDWEIGHTS that fully
hide under the matmul stream, half the SBUF/HBM footprint). PSUM
accumulation is fp32 throughout.

Causal structure: 512-wide q columns; the 4 diagonal k-chunks of each column
only compute the valid suffix (N = 512-128j), with a single [128,128]
triangular mask applied to the first 128 columns of each diagonal chunk.
Score matmuls are emitted up to two k-chunks ahead of the softmax/av chain
so the exp activation latency hides under PE work.

Scalar-engine discipline (the in-order PE otherwise stalls on it at head
boundaries): the scalar engine only ever runs EXP and COPY (one activation
table load total. The gate is stored as exp(-garg) and sigmoid is folded
into the gating denominator, gat = av / (sm*e + sm), evaluated with the
fast approximate reciprocal on DVE. Each head's first projection chunk is
emitted inside the previous head's attention tail (tail_fill), as is the
first o_proj block, so the PE self-fills while scalar drains exps.
"""

import sys

if "/opt/trn_rl_repo" not in sys.path:
    sys.path.insert(0, "/opt/trn_rl_repo")

import ml_dtypes
import numpy as np

import concourse.bass as bass
import concourse.tile as tile
from concourse import bacc, mybir
from concourse.bass_utils import run_bass_kernel_spmd

F32 = mybir.dt.float32
BF16 = mybir.dt.bfloat16
AF = mybir.ActivationFunctionType

S = 2048          # tokens per batch element
E = 2048          # model dim
D = 256           # head dim
RD = 64           # rope dims
NHC = 4           # q heads per core
HD = NHC * D      # per-core head dims (1024)
ECH = E // 128    # 16 contraction chunks
QCH = HD // 128   # 8 per-core q/g/o d-chunks
TT = 4            # 512-wide token tiles
NKC = S // 128    # 16 k chunks
NQC = S // 128    # 16 q chunks (oproj)


def _body(tc, d):
    nc = tc.nc
    ts = bass.ts

    from contextlib import ExitStack

    stack = ExitStack()

    pm = stack.enter_context(tc.tile_pool(name="main", bufs=1))
    kt = pm.tile([128, 2, S], BF16, tag="kt")
    vt = pm.tile([128, NKC, D], BF16, tag="vt")
    gat = pm.tile([128, QCH, S], BF16, tag="gat")
    ones = pm.tile([128, 128], BF16, tag="ones")
    nc.gpsimd.dma_start(ones[:], d["ones"].ap())
    tri = pm.tile([128, 128], BF16, tag="tri")
    nc.gpsimd.dma_start(tri[:], d["tri"].ap())
    rotm = pm.tile([RD, RD], BF16, tag="rotm")
    nc.gpsimd.dma_start(rotm[:], d["rotm"].ap())
    cos_t = pm.tile([RD, S], BF16, tag="cos")
    nc.gpsimd.dma_start(cos_t[:], d["cost"].ap())
    sin_t = pm.tile([RD, S], BF16, tag="sin")
    nc.gpsimd.dma_start(sin_t[:], d["sint"].ap())

    p_qg = pm
    p_exp = pm
    p_gt2 = pm
    p_rtmp = pm
    psum = stack.enter_context(tc.tile_pool(name="psum", bufs=8, space="PSUM"))

    def rope(dst, t):
        # dst: [128, 512] SBUF slice whose partitions 0:RD hold rope dims.
        # rot = R @ x via PE, then dst[0:RD] = x*cos + rot*sin.
        rp = psum.tile([RD, 512], F32, tag="ps")
        nc.tensor.matmul(rp[:], rotm[:], dst[0:RD, :], start=True, stop=True)
        tmp = p_rtmp.tile([RD, 512], F32, tag="rt", bufs=2)
        nc.vector.tensor_mul(tmp[:], dst[0:RD, :], cos_t[:, ts(t, 512)])
        nc.vector.tensor_mul(dst[0:RD, :], rp[:], sin_t[:, ts(t, 512)])
        nc.vector.tensor_add(dst[0:RD, :], dst[0:RD, :], tmp[:])

    def attention(h, q_h, g_h, tail_fill=None):
        tasks = [(qq, kk) for qq in range(TT) for kk in range(4 * qq + 4)]
        sps = {}
        col = {}

        def emit_sp(qq, kk):
            j = kk - 4 * qq
            off = 128 * j if j > 0 else 0
            w = 512 - off
            spt = psum.tile([128, 512], F32, tag="ps")
            sp = spt[:, 0:w]
            base = 512 * qq + off
            nc.tensor.matmul(
                sp, kt[:, 0, ts(kk, 128)], q_h[:, 0, base : 512 * qq + 512],
                start=True, stop=False,
            )
            nc.tensor.matmul(
                sp, kt[:, 1, ts(kk, 128)], q_h[:, 1, base : 512 * qq + 512],
                start=False, stop=True,
            )
            sps[(qq, kk)] = (sp, off, w)

        for i0 in range(3):
            emit_sp(*tasks[i0])
        emitted = 3
        for idx, (qq, kk) in enumerate(tasks):
            # keep the score pipeline 2 deep mid-column (hides exp latency on
            # the narrow diagonal chunks); drop to 1 at column boundaries so
            # PSUM never exceeds 8 live banks.
            while emitted < len(tasks) and (
                emitted == idx + 1
                or (emitted == idx + 2 and tasks[emitted][0] == qq and kk > 0)
            ):
                emit_sp(*tasks[emitted])
                emitted += 1
            if idx + 1 == len(tasks) and tail_fill is not None:
                # next head's first projection chunk: PE fills the pipe while
                # the scalar engine drains the last exp activations.
                tail_fill()
            sp, off, w = sps.pop((qq, kk))
            ext = p_exp.tile([128, 512], BF16, tag="ex", bufs=6)
            ex = ext[:, 0:w]
            nc.scalar.activation(ex, sp, AF.Exp, scale=0.0625)
            if kk - 4 * qq >= 0:
                nc.vector.tensor_mul(ex[:, 0:128], ex[:, 0:128], tri[:])
            if kk == 0:
                col[qq] = (
                    psum.tile([128, 512], F32, tag="ps", name="av0"),
                    psum.tile([128, 512], F32, tag="ps", name="av1"),
                    psum.tile([128, 512], F32, tag="ps", name="sm"),
                )
            av0, av1, sm = col[qq]
            st, en = (kk == 0), (kk == 4 * qq + 3)
            nc.tensor.matmul(av0[:, off:512], vt[:, kk, 0:128], ex, start=st, stop=en)
            nc.tensor.matmul(av1[:, off:512], vt[:, kk, 128:256], ex, start=st, stop=en)
            nc.tensor.matmul(sm[:, off:512], ones[:], ex, start=st, stop=en)
            if en:
                # g_h holds exp(-garg); fold the gate and softmax
                # denominators: gat = av / (sm*(1+e^-g)) = av / (sm*e + sm)
                # via one fast approximate reciprocal (~18 bits) per half.
                for c, avc in ((0, av0), (1, av1)):
                    den = p_gt2.tile([128, 512], F32, tag="gtmp", bufs=2)
                    nc.vector.tensor_mul(den[:], sm[:], g_h[:, c, ts(qq, 512)])
                    nc.vector.tensor_add(den[:], den[:], sm[:])
                    rec = p_gt2.tile([128, 512], F32, tag="rec", bufs=2)
                    nc.vector.reciprocal_approx_fast(rec[:], den[:])
                    with nc.allow_low_precision(reason="bf16 gated-attn by design"):
                        nc.vector.tensor_mul(
                            gat[:, 2 * h + c, ts(qq, 512)], avc[:], rec[:]
                        )
                del col[qq]

    # ---------------- Phase A: projections + attention (xt resident) --------
    last_qg = None
    with (
        tc.tile_pool(name="xt", bufs=1) as p_xt,
        tc.tile_pool(name="w", bufs=3) as p_w,
        tc.tile_pool(name="wv", bufs=1) as p_wv,
    ):
        # DMA schedule: the PE's first work is v(ec=0) then k(ec=0), so land
        # xt[ec0] (scalar queue, ahead of everything), then wv/wk quarters
        # interleaved with even xt chunks on sync; odd xt chunks on scalar.
        # All three queues are hardware-DGE.  wq prefetches ride on sync.
        wv_t = p_wv.tile([128, ECH, D], BF16, tag="wv")
        wkt = p_wv.tile([128, 2, ECH, 128], BF16, tag="wk")
        xt = p_xt.tile([128, ECH, S], BF16, tag="xt")

        # first chunk in 512-column quarters so ec0's matmuls start with the
        # first 128KB instead of waiting for the whole 512KB transfer
        for t in range(TT):
            nc.scalar.dma_start(
                xt[:, 0, ts(t, 512)], d["xt"].ap()[:, 0, ts(t, 512)]
            )
        for wh in range(4):
            nc.sync.dma_start(
                wkt[:, 0, ts(wh, 4), :], d["wk"].ap()[0][:, ts(wh, 4), :]
            )
            nc.sync.dma_start(
                wkt[:, 1, ts(wh, 4), :], d["wk"].ap()[1][:, ts(wh, 4), :]
            )
            nc.sync.dma_start(xt[:, 2 + 2 * wh, :], d["xt"].ap()[:, 2 + 2 * wh, :])
        for ec in (1, 3, 5, 7, 9, 11, 13, 15):
            nc.scalar.dma_start(xt[:, ec, :], d["xt"].ap()[:, ec, :])
        for ec in (10, 12, 14):
            nc.sync.dma_start(xt[:, ec, :], d["xt"].ap()[:, ec, :])
        # wv lands after the k-pass xt chunks: v's passes only start ~27us in
        for wh in range(4):
            nc.sync.dma_start(
                wv_t[:, ts(wh, 4), :], d["wv"].ap()[:, ts(wh, 4), :]
            )

        # both k chunks first, ec-outer (8 PSUM banks, ~1.7us of PE burn per
        # 512KB xt chunk): matches the DMA arrival rate during the ramp.
        # PSUM accumulation is commutative, so consume ec chunks in their
        # two-queue arrival order (scalar: 0,1,3,..; sync: 2,4,..) instead of
        # numerically — the in-order PE never blocks on a late early chunk.
        ECS = [0, 2, 1, 4, 3, 6, 5, 8, 7, 10, 9, 12, 11, 14, 13, 15]
        kss = []
        for p in range(2):
            for t in range(TT):
                pk = psum.tile([128, 512], F32, tag="ps", name="pk")
                kss.append(pk)
        for pos, ec in enumerate(ECS):
            # t-major: each arriving 512-column quarter of an xt chunk
            # unlocks both k-chunk matmuls immediately
            for t in range(TT):
                for p in range(2):
                    nc.tensor.matmul(
                        kss[4 * p + t][:],
                        wkt[:, p, ec, :],
                        xt[:, ec, ts(t, 512)],
                        start=(pos == 0),
                        stop=(pos == ECH - 1),
                    )
        for p in range(2):
            for t in range(TT):
                nc.scalar.copy(kt[:, p, ts(t, 512)], kss[4 * p + t][:])

        # v token-chunks, 8 banks at a time
        for wave in range(2):
            pss = []
            for i in range(8):
                pv = psum.tile([128, D], F32, tag="ps", name="pv")
                pss.append(pv)
            for ec in range(ECH):
                for i in range(8):
                    nc.tensor.matmul(
                        pss[i][:],
                        xt[:, ec, ts(8 * wave + i, 128)],
                        wv_t[:, ec, :],
                        start=(ec == 0),
                        stop=(ec == ECH - 1),
                    )
            for i in range(8):
                nc.scalar.copy(vt[:, 8 * wave + i, :], pss[i][:])

        # k rope here: its PSUM->SBUF copies completed during the v tail, so
        # the rp matmuls never wait on the scalar engine.
        for t in range(TT):
            rope(kt[:, 0, ts(t, 512)], t)

        def fetch_w(w_ap):
            wt = p_w.tile([128, ECH, 128], BF16, tag="w", name="wt")
            nc.sync.dma_start(wt[:], w_ap)
            return wt

        def proj_mm(wt, tiles=tuple(range(TT))):
            pss = []
            for t in tiles:
                pt = psum.tile([128, 512], F32, tag="ps")
                pss.append(pt)
            for ec in range(ECH):
                for i, t in enumerate(tiles):
                    nc.tensor.matmul(
                        pss[i][:],
                        wt[:, ec, :],
                        xt[:, ec, ts(t, 512)],
                        start=(ec == 0),
                        stop=(ec == ECH - 1),
                    )
            return pss

        def proj_fin(pss, dst, dst_idx, kind, tiles=tuple(range(TT))):
            for i, t in enumerate(tiles):
                dslice = dst[:, dst_idx, ts(t, 512)]
                if kind == "g":
                    # store exp(-garg): the scalar engine stays on the EXP
                    # table (no 1.3us reloads); sigmoid is folded into the
                    # gating denominator as sm*(1+e) = sm*e + sm.
                    nc.scalar.activation(dslice, pss[i][:], AF.Exp, scale=-1.0)
                elif kind == "q" and dst_idx == 1:
                    # q chunk 1 finishes right before attention: route its
                    # copies to the vector engine so the scalar queue is
                    # already clear for attention's first exp.
                    with nc.allow_low_precision(reason="bf16 q storage by design"):
                        nc.vector.tensor_scalar_add(dslice, pss[i][:], 0.0)
                else:
                    nc.scalar.copy(dslice, pss[i][:])

        def proj_chunk(w_ap, dst, dst_idx, kind, wt=None):
            if wt is None:
                wt = fetch_w(w_ap)
            proj_fin(proj_mm(wt), dst, dst_idx, kind)

        # Each head's first weight chunk is prefetched on the sync queue and
        # its 64 projection matmuls are emitted inside the previous head's
        # attention tail (tail_fill), so the PE never idles at head
        # boundaries while the scalar engine drains exp activations.
        def make_qg():
            q_h = p_qg.tile([128, 2, S], BF16, tag="qh", bufs=2)
            g_h = p_qg.tile([128, 2, S], BF16, tag="gh", bufs=2)
            return q_h, g_h

        # Per-head order g0,g1,q0,rope,q1: the gate exps land early on the
        # scalar queue, so it is empty when attention's first exp arrives.
        cur_qg = make_qg()
        pending = [proj_mm(fetch_w(d["wg"].ap()[0]))]
        for h in range(NHC):
            q_h, g_h = cur_qg
            proj_fin(pending.pop(), g_h, 0, "g")
            proj_chunk(d["wg"].ap()[2 * h + 1], g_h, 1, "g")
            proj_chunk(d["wq"].ap()[2 * h], q_h, 0, "q")
            for t in range(TT):
                rope(q_h[:, 0, ts(t, 512)], t)
            # q1 in two 2-bank halves: tiles 0-1 (used by attention columns
            # 0-1) finish and copy ~7us before the head boundary, so only two
            # copies share the vector queue with attention's first mask.
            wt1 = fetch_w(d["wq"].ap()[2 * h + 1])
            pss_a = proj_mm(wt1, tiles=(0, 1))
            proj_fin(pss_a, q_h, 1, "q", tiles=(0, 1))
            pss_b = proj_mm(wt1, tiles=(2, 3))
            proj_fin(pss_b, q_h, 1, "q", tiles=(2, 3))
            if h < NHC - 1:
                cur_qg = make_qg()
                pre = fetch_w(d["wg"].ap()[2 * h + 2])

                def tail_fill(wt=pre):
                    pending.append(proj_mm(wt))

                attention(h, q_h, g_h, tail_fill=tail_fill)
            else:
                last_qg = (q_h, g_h)

    # ---------------- Phase B: last head's attention + o_proj ---------------
    with (
        tc.tile_pool(name="wo", bufs=1) as p_wo,
        tc.tile_pool(name="ob", bufs=4) as p_ob,
    ):
        wo_t = p_wo.tile([128, QCH, E], BF16, tag="wo")
        for i in range(4):
            nc.sync.dma_start(
                wo_t[:, 2 * i : 2 * i + 2, :], d["wo"].ap()[:, 2 * i : 2 * i + 2, :]
            )

        def oproj_mm(qc):
            ops = []
            for et in range(4):
                op = psum.tile([128, 512], F32, tag="ps", name="op")
                ops.append(op)
            for hc in range(QCH):
                gd = gat[:, hc, ts(qc, 128)]
                for et in range(4):
                    nc.tensor.matmul(
                        ops[et][:],
                        gd,
                        wo_t[:, hc, ts(et, 512)],
                        start=(hc == 0),
                        stop=(hc == QCH - 1),
                    )
            return ops

        pend_o = []

        def tail_fill3():
            pend_o.append(oproj_mm(0))

        attention(NHC - 1, *last_qg, tail_fill=tail_fill3)

        for qc in range(NQC):
            ops = pend_o.pop() if qc == 0 else oproj_mm(qc)
            ob = p_ob.tile([128, E], BF16, tag="ob")
            for et in range(4):
                nc.scalar.copy(ob[:, ts(et, 512)], ops[et][:])
                nc.sync.dma_start(
                    d["out"].ap()[qc][:, ts(et, 512)], ob[:, ts(et, 512)]
                )

    stack.close()


def build_nc():
    nc = bacc.Bacc("TRN2", target_bir_lowering=False, debug=False)
    d = {}
    d["xt"] = nc.dram_tensor("xt", [128, ECH, S], BF16, kind="ExternalInput")
    d["wq"] = nc.dram_tensor("wq", [QCH, 128, ECH, 128], BF16, kind="ExternalInput")
    d["wg"] = nc.dram_tensor("wg", [QCH, 128, ECH, 128], BF16, kind="ExternalInput")
    d["wk"] = nc.dram_tensor("wk", [2, 128, ECH, 128], BF16, kind="ExternalInput")
    d["wv"] = nc.dram_tensor("wv", [128, ECH, D], BF16, kind="ExternalInput")
    d["wo"] = nc.dram_tensor("wo", [128, QCH, E], BF16, kind="ExternalInput")
    d["cost"] = nc.dram_tensor("cost", [RD, S], BF16, kind="ExternalInput")
    d["sint"] = nc.dram_tensor("sint", [RD, S], BF16, kind="ExternalInput")
    d["tri"] = nc.dram_tensor("tri", [128, 128], BF16, kind="ExternalInput")
    d["rotm"] = nc.dram_tensor("rotm", [RD, RD], BF16, kind="ExternalInput")
    d["ones"] = nc.dram_tensor("ones", [128, 128], BF16, kind="ExternalInput")
    d["out"] = nc.dram_tensor("out", [NQC, 128, E], BF16, kind="ExternalOutput")
    with tile.TileContext(nc) as tc:
        _body(tc, d)
    nc.compile()
    return nc


_NC_CACHE = None


def _get_nc():
    global _NC_CACHE
    if _NC_CACHE is None:
        _NC_CACHE = build_nc()
    return _NC_CACHE


def _rope_tables():
    inv = 1.0 / (10000.0 ** (np.arange(0, RD, 2, dtype=np.float32) / np.float32(RD)))
    t = np.arange(S, dtype=np.float32)
    freqs = np.outer(t, inv).astype(np.float32)          # [S, RD/2]
    emb = np.concatenate([freqs, freqs], axis=1)         # [S, RD]
    return (
        np.ascontiguousarray(np.cos(emb).T).astype(ml_dtypes.bfloat16),
        np.ascontiguousarray(np.sin(emb).T).astype(ml_dtypes.bfloat16),
    )


def _rotm():
    r = np.zeros((RD, RD), dtype=np.float32)  # r[j, d] = R[d, j], rot = R @ x
    half = RD // 2
    for dd in range(half):
        r[dd + half, dd] = -1.0
    for dd in range(half, RD):
        r[dd - half, dd] = 1.0
    return r.astype(ml_dtypes.bfloat16)


def _tri():
    k = np.arange(128)[:, None]
    q = np.arange(128)[None, :]
    return (k <= q).astype(ml_dtypes.bfloat16)


def _prep_in_maps(hidden_states, Wq, Wk, Wv, Wg, Wo):
    cosT, sinT = _rope_tables()
    tri = _tri()
    rotm = _rotm()
    ones = np.ones((128, 128), dtype=ml_dtypes.bfloat16)
    maps = []
    for c in range(8):
        b, t = c // 4, c % 4
        hq0, kvh = 4 * t, (t // 2)
        cols = slice(hq0 * D, (hq0 + NHC) * D)
        kcols = slice(kvh * D, (kvh + 1) * D)
        x = hidden_states[b]  # [S, E]
        m = {
            "xt": np.ascontiguousarray(
                x.T.reshape(ECH, 128, S).transpose(1, 0, 2)
            ).astype(ml_dtypes.bfloat16),
            "wq": np.ascontiguousarray(
                Wq[:, cols].reshape(ECH, 128, QCH, 128).transpose(2, 1, 0, 3)
            ).astype(ml_dtypes.bfloat16),
            "wg": np.ascontiguousarray(
                Wg[:, cols].reshape(ECH, 128, QCH, 128).transpose(2, 1, 0, 3)
            ).astype(ml_dtypes.bfloat16),
            "wk": np.ascontiguousarray(
                Wk[:, kcols].reshape(ECH, 128, 2, 128).transpose(2, 1, 0, 3)
            ).astype(ml_dtypes.bfloat16),
            "wv": np.ascontiguousarray(
                Wv[:, kcols].reshape(ECH, 128, D).transpose(1, 0, 2)
            ).astype(ml_dtypes.bfloat16),
            "wo": np.ascontiguousarray(
                Wo[cols, :].reshape(QCH, 128, E).transpose(1, 0, 2)
            ).astype(ml_dtypes.bfloat16),
            "cost": cosT,
            "sint": sinT,
            "tri": tri,
            "rotm": rotm,
            "ones": ones,
        }
        maps.append(m)
    return maps


def _run(inputs, trace=False, trace_cores=None, tmpdir=None):
    nc = _get_nc()
    in_maps = _prep_in_maps(**inputs)
    kw = {}
    if trace:
        kw = dict(trace=True, trace_cores=trace_cores, tmpdir=tmpdir)
    res = run_bass_kernel_spmd(nc, in_maps, list(range(8)), **kw)
    outs = [
        np.asarray(res.results[c]["out"], dtype=np.float32).reshape(S, E)
        for c in range(8)
    ]
    full = np.stack(
        [
            outs[0] + outs[1] + outs[2] + outs[3],
            outs[4] + outs[5] + outs[6] + outs[7],
        ]
    ).astype(np.float32)
    return full, res


def kernel(hidden_states, Wq, Wk, Wv, Wg, Wo):
    full, _ = _run(
        dict(hidden_states=np.asarray(hidden_states, dtype=np.float32),
             Wq=np.asarray(Wq, dtype=np.float32),
             Wk=np.asarray(Wk, dtype=np.float32),
             Wv=np.asarray(Wv, dtype=np.float32),
             Wg=np.asarray(Wg, dtype=np.float32),
             Wo=np.asarray(Wo, dtype=np.float32))
    )
    return full


if __name__ == "__main__":
    nc = build_nc()
    print("build OK")
